# revision 1
# baseline (speedup 1.0000x reference)
"""Trainium2 Bass kernel for nn_Knowledge_Decomposition.

Computation (per reference):
  g_spec = MLP_gs(gfeat);  p_spec = MLP_ps(pfeat)
  common = Interaction(a=pfeat, b=gfeat; c_* params)
  synergy = Interaction(a=pfeat, b=gfeat; s_* params)
where MLP(x) = relu(LN(x @ W.T + b) * g + beta) and Interaction computes
  g_align = MLP_g(a), p_align = MLP_p(b)
  out = p_align * sigmoid(p_align * <g_align, awp> + abp)
      + g_align * sigmoid(g_align * <p_align, awg> + abg)

Sharding: pure data parallel. B=128 rows split across 8 cores (16 rows,
i.e. 256 tokens of dim 256 per core); params replicated.

Layout on core: tokens on SBUF partitions (2 chunks of 128), features on
the free dim. x is PE-transposed once per core so the contraction dim
feeds the matmul partitions; weights are pre-transposed on the host.
"""

import sys

if "/opt/trn_rl_repo" not in sys.path:
    sys.path.insert(0, "/opt/trn_rl_repo")

import numpy as np

import concourse.bacc as bacc
import concourse.bass as bass
from concourse import mybir
from concourse.masks import make_identity
from concourse.tile import TileContext
from concourse.bass_utils import run_bass_kernel_spmd

AF = mybir.ActivationFunctionType
ALU = mybir.AluOpType
F32 = mybir.dt.float32

N_CORES = 8
B, L, D = 128, 16, 256
BS = B // N_CORES          # batch rows per core
T = BS * L                 # tokens per core = 256
P = 128                    # SBUF partitions
NT = T // P                # token chunks per core = 2
NK = D // P                # contraction chunks = 2
LN_EPS = 1e-5

MLPS = ["gs", "ps", "c_g", "c_p", "s_g", "s_p"]
# which transposed input feeds each MLP ('g' = gfeat, 'p' = pfeat).
# NOTE: reference calls interaction(a=pfeat, bfeat=gfeat): the *_g MLPs
# (g_align) consume pfeat and the *_p MLPs (p_align) consume gfeat.
MLP_INPUT = {"gs": "g", "ps": "p", "c_g": "p", "c_p": "g", "s_g": "p", "s_p": "g"}
MLP_BY_INP = {"g": ["gs", "c_p", "s_p"], "p": ["ps", "c_g", "s_g"]}
# LN processing order: interaction inputs first so interactions start early
LN_ORDER = ["c_g", "c_p", "s_g", "s_p", "gs", "ps"]
AW_KEYS = ["c_g", "c_p", "s_g", "s_p"]  # c_g<-c_agw, c_p<-c_apw, ...


def _bcast_rows(ap, p):
    """Broadcast a [N] DRAM AP across p partitions -> [p, N] (stride-0)."""
    return bass.AP(tensor=ap.tensor, offset=ap.offset, ap=[[0, p]] + list(ap.ap))


def _build(affine_identity: bool, ab: dict[str, float]):
    """Build + compile the per-core Bass program (SPMD; same on all cores)."""
    nc = bacc.Bacc("TRN2", target_bir_lowering=False, debug=False)

    xg = nc.dram_tensor("xg", [T, D], F32, kind="ExternalInput")
    xp = nc.dram_tensor("xp", [T, D], F32, kind="ExternalInput")
    xin = {"g": xg, "p": xp}
    wt_d = {m: nc.dram_tensor(f"wt_{m}", [D, D], F32, kind="ExternalInput") for m in MLPS}
    aw_d = {k: nc.dram_tensor(f"aw_{k}", [P, D], F32, kind="ExternalInput") for k in AW_KEYS}
    if not affine_identity:
        b_d = {m: nc.dram_tensor(f"b_{m}", [D], F32, kind="ExternalInput") for m in MLPS}
        g_d = {m: nc.dram_tensor(f"g_{m}", [D], F32, kind="ExternalInput") for m in MLPS}
        bt_d = {m: nc.dram_tensor(f"bt_{m}", [D], F32, kind="ExternalInput") for m in MLPS}
    outs = {
        name: nc.dram_tensor(name, [T, D], F32, kind="ExternalOutput")
        for name in ["o_common", "o_synergy", "o_gspec", "o_pspec"]
    }

    with TileContext(nc) as tc:
        with (
            tc.tile_pool(name="consts", bufs=1) as consts,
            tc.tile_pool(name="xnat", bufs=4) as xnat,
            tc.tile_pool(name="work", bufs=14) as work,
            tc.tile_pool(name="spool", bufs=14) as spool,
            tc.tile_pool(name="tpsum", bufs=2, space="PSUM") as tpsum,
            tc.tile_pool(name="hpsum", bufs=6, space="PSUM") as hpsum,
        ):
            ident = consts.tile([P, P], F32)
            make_identity(nc, ident)
            eps_t = consts.tile([P, 1], F32)
            nc.vector.memset(eps_t[:], LN_EPS)
            abt = {}
            for k in AW_KEYS:
                abt[k] = consts.tile([P, 1], F32, tag=f"ab_{k}", name=f"ab_{k}")
                nc.vector.memset(abt[k][:], ab[k])

            # weights, pre-transposed on host: wt[k, j] = W[j, k]
            wt_t = {}
            for m in MLPS:
                wt_t[m] = consts.tile([P, NK, D], F32, tag=f"wt_{m}", name=f"wt_{m}")
                nc.sync.dma_start(
                    out=wt_t[m][:],
                    in_=wt_d[m][:].rearrange("(kb p) j -> p kb j", p=P),
                )
            # attention weight vectors broadcast across partitions
            awbc = {}
            for k in AW_KEYS:
                awbc[k] = consts.tile([P, D], F32, tag=f"aw_{k}", name=f"aw_{k}")
                nc.sync.dma_start(out=awbc[k][:], in_=aw_d[k][:])

            if not affine_identity:
                ones_t = consts.tile([1, P], F32, tag="ones")
                nc.vector.memset(ones_t[:], 1.0)
                b_t, gbc, btbc = {}, {}, {}
                for m in MLPS:
                    b_t[m] = consts.tile([1, D], F32, tag=f"b_{m}", name=f"b_{m}")
                    nc.sync.dma_start(out=b_t[m][:], in_=b_d[m][:].rearrange("d -> 1 d"))
                    gbc[m] = consts.tile([P, D], F32, tag=f"g_{m}", name=f"g_{m}")
                    nc.gpsimd.dma_start(out=gbc[m][:], in_=_bcast_rows(g_d[m][:], P))
                    btbc[m] = consts.tile([P, D], F32, tag=f"bt_{m}", name=f"bt_{m}")
                    nc.gpsimd.dma_start(out=btbc[m][:], in_=_bcast_rows(bt_d[m][:], P))

            # x loaded naturally ([tok, feat]) then PE-transposed into
            # xt[inp][:, kb, t] = x[t, kb*P + p]  (feature chunks on partitions)
            xt = {}
            for inp in ("g", "p"):
                xt[inp] = consts.tile([P, NK, T], F32, tag=f"xt_{inp}", name=f"xt_{inp}")
                for nb in range(NT):
                    xn = xnat.tile([P, D], F32, tag="xn")
                    nc.sync.dma_start(out=xn[:], in_=xin[inp][nb * P:(nb + 1) * P, :])
                    for kb in range(NK):
                        tp = tpsum.tile([P, P], F32, tag="tp")
                        nc.tensor.transpose(tp[:], xn[:, kb * P:(kb + 1) * P], ident[:])
                        nc.vector.tensor_copy(out=xt[inp][:, kb, nb * P:(nb + 1) * P], in_=tp[:])

            # ---- all matmuls densely, both token chunks into full-bank PSUM ----
            hp = {}
            for inp in ("g", "p"):
                for kb in range(NK):
                    for nb in range(NT):
                        tok = slice(nb * P, (nb + 1) * P)
                        for m in MLP_BY_INP[inp]:
                            if kb == 0 and nb == 0:
                                hp[m] = hpsum.tile([P, NT, D], F32, tag="hp", name=f"hp_{m}")
                            nc.tensor.matmul(
                                hp[m][:, nb, :],
                                lhsT=xt[inp][:, kb, tok],
                                rhs=wt_t[m][:, kb, :],
                                start=(kb == 0 and nb == 0),
                                stop=(kb == NK - 1 and nb == NT - 1 and affine_identity),
                            )
                if not affine_identity:
                    for nb in range(NT):
                        for m in MLP_BY_INP[inp]:
                            nc.tensor.matmul(
                                hp[m][:, nb, :],
                                lhsT=ones_t[0:1, :],
                                rhs=b_t[m][0:1, :],
                                start=False,
                                stop=(nb == NT - 1),
                            )

            # ---- batched LN stats: 12 tiles -> one Sqrt, one reciprocal ----
            TILES = [(m, nb) for nb in range(NT) for m in LN_ORDER]
            mva = spool.tile([P, 12, 2], F32, tag="mva")
            for i, (m, nb) in enumerate(TILES):
                stats = spool.tile([P, 6], F32, tag="stats")
                nc.vector.bn_stats(stats[:], hp[m][:, nb, :])
                nc.vector.bn_aggr(mva[:, i, :], stats[:])
            stdall = spool.tile([P, 12], F32, tag="stdall")
            nc.scalar.activation(stdall[:], mva[:, :, 1], AF.Sqrt, bias=eps_t[:])
            rstdall = spool.tile([P, 12], F32, tag="rstdall")
            nc.vector.reciprocal(rstdall[:], stdall[:])
            nmrall = spool.tile([P, 12], F32, tag="nmrall")
            nc.vector.tensor_mul(nmrall[:], mva[:, :, 0], rstdall[:])
            nc.vector.tensor_scalar(nmrall[:], nmrall[:], scalar1=-1.0, scalar2=None, op0=ALU.mult)

            # ---- normalize+relu (grouped on ACT), aligns first ----
            aligns = {}
            for i, (m, nb) in enumerate(TILES):
                is_align = m not in ("gs", "ps")
                otag = "align" if is_align else "spec"
                ot = work.tile([P, D], F32, tag=otag, name=f"ot_{m}_{nb}")
                if affine_identity:
                    nc.scalar.activation(ot[:], hp[m][:, nb, :], AF.Relu,
                                         bias=nmrall[:, i:i + 1], scale=rstdall[:, i:i + 1])
                else:
                    nc.scalar.activation(ot[:], hp[m][:, nb, :], AF.Identity,
                                         bias=nmrall[:, i:i + 1], scale=rstdall[:, i:i + 1])
                    nc.vector.tensor_mul(ot[:], ot[:], gbc[m][:])
                    nc.vector.tensor_add(ot[:], ot[:], btbc[m][:])
                    nc.vector.tensor_scalar_max(ot[:], ot[:], 0.0)
                tok = slice(nb * P, (nb + 1) * P)
                if m == "gs":
                    nc.sync.dma_start(out=outs["o_gspec"][tok, :], in_=ot[:])
                elif m == "ps":
                    nc.sync.dma_start(out=outs["o_pspec"][tok, :], in_=ot[:])
                else:
                    aligns[(m, nb)] = ot

            # ---- interactions: dots (DVE), sigmoids (ACT, grouped), combine ----
            dots = {}
            for nb in range(NT):
                for pr in ("c", "s"):
                    gal = aligns[(pr + "_g", nb)]
                    pal = aligns[(pr + "_p", nb)]
                    sc1 = work.tile([P, D], F32, tag="ttscratch")
                    dp = spool.tile([P, 1], F32, tag="dp", name=f"dp_{pr}_{nb}")
                    nc.vector.tensor_mul(sc1[:], pal[:], awbc[pr + "_g"][:])
                    nc.vector.tensor_reduce(dp[:], sc1[:], axis=mybir.AxisListType.X, op=ALU.add)
                    sc2 = work.tile([P, D], F32, tag="ttscratch")
                    dg = spool.tile([P, 1], F32, tag="dg", name=f"dg_{pr}_{nb}")
                    nc.vector.tensor_mul(sc2[:], gal[:], awbc[pr + "_p"][:])
                    nc.vector.tensor_reduce(dg[:], sc2[:], axis=mybir.AxisListType.X, op=ALU.add)
                    dots[(pr, nb)] = (dp, dg)
            for nb in range(NT):
                for pr, oname in (("c", "o_common"), ("s", "o_synergy")):
                    gal = aligns[(pr + "_g", nb)]
                    pal = aligns[(pr + "_p", nb)]
                    dp, dg = dots[(pr, nb)]
                    gat = work.tile([P, D], F32, tag="att")
                    nc.scalar.activation(gat[:], gal[:], AF.Sigmoid, bias=abt[pr + "_g"][:], scale=dp[:])
                    pat = work.tile([P, D], F32, tag="att")
                    nc.scalar.activation(pat[:], pal[:], AF.Sigmoid, bias=abt[pr + "_p"][:], scale=dg[:])
                    t1 = work.tile([P, D], F32, tag="t1")
                    nc.gpsimd.tensor_mul(t1[:], pal[:], pat[:])
                    t2 = work.tile([P, D], F32, tag="t2")
                    nc.gpsimd.tensor_mul(t2[:], gal[:], gat[:])
                    ot2 = work.tile([P, D], F32, tag="iout")
                    nc.vector.tensor_add(ot2[:], t1[:], t2[:])
                    tok = slice(nb * P, (nb + 1) * P)
                    nc.sync.dma_start(out=outs[oname][tok, :], in_=ot2[:])

    nc.compile()
    return nc


_CACHE: dict = {}


def _get_program(affine_identity: bool, ab: dict[str, float]):
    key = (affine_identity, tuple(sorted(ab.items())))
    if key not in _CACHE:
        _CACHE[key] = _build(affine_identity, ab)
    return _CACHE[key]


def kernel(**inputs) -> tuple:
    inp = {k: np.asarray(v) for k, v in inputs.items()}
    gfeat = np.ascontiguousarray(inp["gfeat"], dtype=np.float32)
    pfeat = np.ascontiguousarray(inp["pfeat"], dtype=np.float32)

    affine_identity = all(
        (inp[m + "_b"] == 0).all()
        and (inp[m + "_g"] == 1).all()
        and (inp[m + "_beta"] == 0).all()
        for m in MLPS
    )
    ab = {
        "c_g": float(inp["c_agb"]),
        "c_p": float(inp["c_apb"]),
        "s_g": float(inp["s_agb"]),
        "s_p": float(inp["s_apb"]),
    }
    nc = _get_program(affine_identity, ab)

    base = {
        f"wt_{m}": np.ascontiguousarray(inp[f"{m}_W"].T, dtype=np.float32)
        for m in MLPS
    }
    base["aw_c_g"] = np.ascontiguousarray(np.broadcast_to(inp["c_agw"].astype(np.float32), (P, D)))
    base["aw_c_p"] = np.ascontiguousarray(np.broadcast_to(inp["c_apw"].astype(np.float32), (P, D)))
    base["aw_s_g"] = np.ascontiguousarray(np.broadcast_to(inp["s_agw"].astype(np.float32), (P, D)))
    base["aw_s_p"] = np.ascontiguousarray(np.broadcast_to(inp["s_apw"].astype(np.float32), (P, D)))
    if not affine_identity:
        for m in MLPS:
            base[f"b_{m}"] = np.ascontiguousarray(inp[f"{m}_b"], dtype=np.float32)
            base[f"g_{m}"] = np.ascontiguousarray(inp[f"{m}_g"], dtype=np.float32)
            base[f"bt_{m}"] = np.ascontiguousarray(inp[f"{m}_beta"], dtype=np.float32)

    gsh = gfeat.reshape(N_CORES, T, D)
    psh = pfeat.reshape(N_CORES, T, D)
    in_maps = [dict(base, xg=gsh[c], xp=psh[c]) for c in range(N_CORES)]

    res = run_bass_kernel_spmd(nc, in_maps, list(range(N_CORES)))

    def gather(name):
        return np.concatenate(
            [res.results[c][name].reshape(BS, L, D) for c in range(N_CORES)], axis=0
        )

    return (gather("o_common"), gather("o_synergy"), gather("o_gspec"), gather("o_pspec"))



# revision 8
# speedup vs baseline: 1.6166x; 1.6166x over previous
"""Trainium2 Bass kernel for nn_Knowledge_Decomposition (fast bf16 version).

Computation (per reference):
  g_spec = MLP_gs(gfeat);  p_spec = MLP_ps(pfeat)
  common = Interaction(a=pfeat, b=gfeat; c_* params)
  synergy = Interaction(a=pfeat, b=gfeat; s_* params)
where MLP(x) = relu(LN(x @ W.T + b) * g + beta) and Interaction computes
  out = p_align * sigmoid(p_align * <g_align, awp> + abp)
      + g_align * sigmoid(g_align * <p_align, awg> + abg)

Fast path (affine-identity LN, which is what setup_inputs produces):
  - hosts pre-transposes x and converts to bf16; weights are merged into two
    wide bf16 rhs operands and CENTERED on host (rank-1 update folds the LN
    mean subtraction into the matmul: hc = x @ (Wt - colmean(Wt)))
  - relu(LN(h)) = relu(hc)*rstd since rstd > 0, so the 1/std scale is
    deferred into per-token scalars applied with tensor_scalar / ACT-scale
  - rstd = 1/sqrt(var+eps) via Quake-style bit trick + 2 Newton steps on
    GpSimd (avoids ACT Sqrt: no activation-table switch; sigmoid+relu live
    in one table set that is prefetched with a dummy op)
  - var from bn_stats even/odd moments: var*256 = cv_e + cv_o + 64*(mu_e-mu_o)^2

Sharding: pure data parallel. B=128 rows split across 8 cores (16 rows,
256 tokens of dim 256 per core); params replicated.
"""

import sys

if "/opt/trn_rl_repo" not in sys.path:
    sys.path.insert(0, "/opt/trn_rl_repo")

import numpy as np
import ml_dtypes

import concourse.bacc as bacc
import concourse.bass as bass
from concourse import mybir
from concourse.masks import make_identity
from concourse.tile import TileContext
from concourse.bass_utils import run_bass_kernel_spmd

AF = mybir.ActivationFunctionType
ALU = mybir.AluOpType
F32 = mybir.dt.float32
BF16 = mybir.dt.bfloat16
I32 = mybir.dt.int32

N_CORES = 8
B, L, D = 128, 16, 256
BS = B // N_CORES          # batch rows per core
T = BS * L                 # tokens per core = 256
P = 128                    # SBUF partitions
NT = T // P                # token chunks per core = 2
NK = D // P                # contraction chunks = 2
LN_EPS = 1e-5
W3 = 3 * D                 # merged rhs width = 768

MLPS = ["gs", "ps", "c_g", "c_p", "s_g", "s_p"]
# reference calls interaction(a=pfeat, bfeat=gfeat): the *_g MLPs
# (g_align) consume pfeat and the *_p MLPs (p_align) consume gfeat.
MLP_INPUT = {"gs": "g", "ps": "p", "c_g": "p", "c_p": "g", "s_g": "p", "s_p": "g"}
# merged rhs column order per input side; R_all slot order is
# [gs, c_p, s_p, c_g, s_g, ps]; align slots R4 = 1..4 = [c_p, s_p, c_g, s_g]
SLAB_ORDER = {"g": ["gs", "c_p", "s_p"], "p": ["c_g", "s_g", "ps"]}
AW_ORDER = ["c_agw", "s_agw", "c_apw", "s_apw"]  # aligned to R4 slots

RSQRT_MAGIC = 0x5F3759DF


def _build_fast(ab4: tuple[float, float, float, float]):
    """Fast program: requires LN affine identity (b=0, g=1, beta=0).

    ab4: sigmoid biases per R4 slot [c_p, s_p, c_g, s_g] =
         (c_apb, s_apb, c_agb, s_agb).
    """
    nc = bacc.Bacc("TRN2", target_bir_lowering=False, debug=False)

    xt_d = {s: nc.dram_tensor(f"xt_{s}", [P, NK * T], BF16, kind="ExternalInput")
            for s in ("g", "p")}
    rhs_d = {s: nc.dram_tensor(f"rhs_{s}", [P, NK * W3], BF16, kind="ExternalInput")
             for s in ("g", "p")}
    aw_d = nc.dram_tensor("aw4", [P, 4 * D], BF16, kind="ExternalInput")
    outs = {
        name: nc.dram_tensor(name, [T, D], F32, kind="ExternalOutput")
        for name in ["o_common", "o_synergy", "o_gspec", "o_pspec"]
    }

    with TileContext(nc) as tc:
        with (
            tc.tile_pool(name="consts", bufs=1) as consts,
            tc.tile_pool(name="work", bufs=1) as work,
            tc.tile_pool(name="hpsum", bufs=1, space="PSUM") as hpsum,
        ):
            # ---- constants / table prefetch / preheat ----
            dmy_in = consts.tile([P, 1], F32, tag="dmy_in")
            nc.vector.memset(dmy_in[:], 0.0)
            dmy_out = consts.tile([P, 1], BF16, tag="dmy_out")
            # prefetch the sigmoid_and_others table (has relu+sigmoid) early
            nc.scalar.activation(dmy_out[:], dmy_in[:], AF.Sigmoid)

            magic = consts.tile([P, 6], I32, tag="magic")
            nc.gpsimd.memset(magic[:], RSQRT_MAGIC)
            # const tiles for the TT-only (gpsimd-legal) rsqrt chain
            cT = {}
            for cname, cval in (("c64", 64.0), ("eps256", 256.0 * LN_EPS),
                                ("cm20", -20.0), ("c30", 30.0), ("c6", 6.0)):
                cT[cname] = consts.tile([P, 6], F32, tag=f"c_{cname}",
                                        name=f"c_{cname}")
                nc.gpsimd.memset(cT[cname][:], cval)

            pre_t = consts.tile([P, 512], BF16, tag="pre")
            nc.vector.memset(pre_t[:], 0.0)

            # ---- input DMAs ----
            xt_t, rhs_t = {}, {}
            for s in ("g", "p"):
                xt_t[s] = consts.tile([P, NK, T], BF16, tag=f"xt_{s}", name=f"xt_{s}")
                nc.sync.dma_start(
                    out=xt_t[s][:],
                    in_=xt_d[s][:].rearrange("p (kb t) -> p kb t", kb=NK),
                )
                rhs_t[s] = consts.tile([P, NK, W3], BF16, tag=f"rhs_{s}", name=f"rhs_{s}")
                for kb in range(NK):
                    nc.sync.dma_start(
                        out=rhs_t[s][:, kb, :],
                        in_=rhs_d[s][:, kb * W3:(kb + 1) * W3],
                    )
            aw4_t = consts.tile([P, 4, D], BF16, tag="aw4")
            nc.sync.dma_start(
                out=aw4_t[:], in_=aw_d[:].rearrange("p (s d) -> p s d", s=4)
            )

            # ---- PSUM slabs + matmuls ----
            # preheat PE (HAM warmup) with dummy matmuls during the DMA wait
            hp = {}
            for nb in range(NT):
                for s in ("g", "p"):
                    hp[(s, nb)] = hpsum.tile([P, W3], F32, tag=f"hp_{s}{nb}",
                                             name=f"hp_{s}{nb}")
            for i in range(5):
                nc.tensor.matmul(
                    hp[("g", 0)][:, 0:512],
                    lhsT=pre_t[:, 0:P],
                    rhs=pre_t[:, 0:512],
                    start=True, stop=True, skip_group_check=True,
                )

            for nb in range(NT):
                tok = slice(nb * P, (nb + 1) * P)
                for s in ("g", "p"):
                    # column-group-major so align columns finish first
                    for (c0, c1) in ((0, 512), (512, W3)):
                        for kb in range(NK):
                            nc.tensor.matmul(
                                hp[(s, nb)][:, c0:c1],
                                lhsT=xt_t[s][:, kb, tok],
                                rhs=rhs_t[s][:, kb, c0:c1],
                                start=(kb == 0),
                                stop=(kb == NK - 1),
                                skip_group_check=True,
                            )

            # ---- per-slab: stats (DVE) + relu (ACT) ----
            S = {}
            R = {}
            for nb in range(NT):
                S[nb] = work.tile([P, 6, 6], F32, tag=f"S{nb}", name=f"S{nb}")
                R[nb] = work.tile([P, 6, D], BF16, tag=f"R{nb}", name=f"R{nb}")
            for nb in range(NT):
                for si, s in enumerate(("g", "p")):
                    base = 3 * si
                    slab = hp[(s, nb)]
                    for j in range(3):
                        nc.vector.bn_stats(
                            S[nb][:, base + j, :], slab[:, j * D:(j + 1) * D]
                        )
                    nc.scalar.activation(
                        R[nb][:, base:base + 3, :],
                        slab[:].rearrange("p (m x) -> p m x", m=3),
                        AF.Relu,
                    )

            # ---- per-nb tail ----
            for nb in range(NT):
                tok = slice(nb * P, (nb + 1) * P)
                Sn, Rn = S[nb], R[nb]
                R4 = Rn[:, 1:5, :]

                # variance from bn_stats moments (GPSIMD, TT-only tiny ops):
                # v256 = var*256 + 256*eps = cv_e + cv_o + 64*(mu_e-mu_o)^2
                #                            + 256*eps
                # rstd = 1/sqrt(var+eps) = 16/sqrt(v256)
                d6 = work.tile([P, 6], F32, tag=f"d6_{nb}", name=f"d6_{nb}")
                nc.gpsimd.tensor_sub(d6[:], Sn[:, :, 1], Sn[:, :, 4])
                dd = work.tile([P, 6], F32, tag=f"dd_{nb}", name=f"dd_{nb}")
                nc.gpsimd.tensor_mul(dd[:], d6[:], d6[:])
                nc.gpsimd.tensor_mul(dd[:], dd[:], cT["c64"][:])
                v2 = work.tile([P, 6], F32, tag=f"v2_{nb}", name=f"v2_{nb}")
                nc.gpsimd.tensor_add(v2[:], Sn[:, :, 2], Sn[:, :, 5])
                nc.gpsimd.tensor_add(v2[:], v2[:], cT["eps256"][:])
                vf = work.tile([P, 6], F32, tag=f"vf_{nb}", name=f"vf_{nb}")
                nc.gpsimd.tensor_add(vf[:], v2[:], dd[:])

                # 16*rsqrt(vf) via bit trick + one 3rd-order Householder step
                # (x16 folded into the polynomial: y0*(30 - 20w + 6w^2)/16
                #  where w = vf*y0^2 ... poly constants already scaled by 16)
                sh = work.tile([P, 6], I32, tag=f"sh_{nb}", name=f"sh_{nb}")
                nc.vector.tensor_scalar(
                    sh[:], vf[:].bitcast(I32), 1, None, op0=ALU.arith_shift_right
                )
                y0 = work.tile([P, 6], I32, tag=f"y0_{nb}", name=f"y0_{nb}")
                nc.gpsimd.tensor_sub(y0[:], magic[:], sh[:])
                y = y0[:].bitcast(F32)
                t1 = work.tile([P, 6], F32, tag=f"t1_{nb}", name=f"t1_{nb}")
                w6 = work.tile([P, 6], F32, tag=f"w6_{nb}", name=f"w6_{nb}")
                w2 = work.tile([P, 6], F32, tag=f"w2_{nb}", name=f"w2_{nb}")
                pa = work.tile([P, 6], F32, tag=f"pa_{nb}", name=f"pa_{nb}")
                rstd = work.tile([P, 6], F32, tag=f"rstd_{nb}", name=f"rstd_{nb}")
                nc.gpsimd.tensor_mul(t1[:], y, y)
                nc.gpsimd.tensor_mul(w6[:], vf[:], t1[:])
                nc.gpsimd.tensor_mul(w2[:], w6[:], w6[:])
                nc.gpsimd.tensor_mul(pa[:], w6[:], cT["cm20"][:])
                nc.gpsimd.tensor_add(pa[:], pa[:], cT["c30"][:])
                nc.gpsimd.tensor_mul(w2[:], w2[:], cT["c6"][:])
                nc.gpsimd.tensor_add(pa[:], pa[:], w2[:])
                nc.gpsimd.tensor_mul(rstd[:], y, pa[:])

                # A4 = R4 * rstd (DVE tensor_scalar, per-token scalar)
                A4 = work.tile([P, 4, D], BF16, tag=f"A4_{nb}", name=f"A4_{nb}")
                for sl in range(4):
                    nc.vector.tensor_scalar(
                        A4[:, sl, :], R4[:, sl, :], rstd[:, 1 + sl:2 + sl], None,
                        op0=ALU.mult,
                    )

                # dots d4[slot] = sum_d A4[slot]*aw4[slot]  (true dots)
                P4 = work.tile([P, 4, D], BF16, tag=f"P4_{nb}", name=f"P4_{nb}")
                nc.vector.tensor_mul(P4[:], A4[:], aw4_t[:])
                d4 = work.tile([P, 4], F32, tag=f"d4_{nb}", name=f"d4_{nb}")
                nc.vector.tensor_reduce(
                    d4[:], P4[:], axis=mybir.AxisListType.X, op=ALU.add
                )

                # att[slot] = sigmoid(A4[slot]*d4[partner] + ab4[slot])
                att4 = work.tile([P, 4, D], BF16, tag=f"att4_{nb}", name=f"att4_{nb}")
                for sl in range(4):
                    pr = (sl + 2) % 4
                    nc.scalar.activation(
                        att4[:, sl, :], A4[:, sl, :], AF.Sigmoid,
                        bias=float(ab4[sl]), scale=d4[:, pr:pr + 1],
                    )

                # U4 = A4*att4 ; common = U0+U2 ; synergy = U1+U3
                U4 = work.tile([P, 4, D], BF16, tag=f"U4_{nb}", name=f"U4_{nb}")
                nc.vector.tensor_mul(U4[:], A4[:], att4[:])
                out2 = work.tile([P, 2, D], F32, tag=f"out2_{nb}", name=f"out2_{nb}")
                nc.gpsimd.tensor_add(out2[:], U4[:, 0:2, :], U4[:, 2:4, :])
                nc.sync.dma_start(out=outs["o_common"][tok, :], in_=out2[:, 0, :])
                nc.sync.dma_start(out=outs["o_synergy"][tok, :], in_=out2[:, 1, :])

                # specs: o_gspec = R[0]*rstd[0]; o_pspec = R[5]*rstd[5]
                gsp = work.tile([P, D], F32, tag=f"gsp_{nb}", name=f"gsp_{nb}")
                nc.vector.tensor_scalar(
                    gsp[:], Rn[:, 0, :], rstd[:, 0:1], None, op0=ALU.mult
                )
                nc.sync.dma_start(out=outs["o_gspec"][tok, :], in_=gsp[:])
                psp = work.tile([P, D], F32, tag=f"psp_{nb}", name=f"psp_{nb}")
                nc.vector.tensor_scalar(
                    psp[:], Rn[:, 5, :], rstd[:, 5:6], None, op0=ALU.mult
                )
                nc.sync.dma_start(out=outs["o_pspec"][tok, :], in_=psp[:])

    nc.compile()
    return nc


# ---------------------------------------------------------------------------
# generic fallback (previous working version): handles arbitrary LN affine
# ---------------------------------------------------------------------------

MLP_BY_INP = {"g": ["gs", "c_p", "s_p"], "p": ["ps", "c_g", "s_g"]}
LN_ORDER = ["c_g", "c_p", "s_g", "s_p", "gs", "ps"]
AW_KEYS = ["c_g", "c_p", "s_g", "s_p"]


def _bcast_rows(ap, p):
    return bass.AP(tensor=ap.tensor, offset=ap.offset, ap=[[0, p]] + list(ap.ap))


def _build_generic(ab: dict[str, float]):
    nc = bacc.Bacc("TRN2", target_bir_lowering=False, debug=False)

    xg = nc.dram_tensor("xg", [T, D], F32, kind="ExternalInput")
    xp = nc.dram_tensor("xp", [T, D], F32, kind="ExternalInput")
    xin = {"g": xg, "p": xp}
    wt_d = {m: nc.dram_tensor(f"wt_{m}", [D, D], F32, kind="ExternalInput") for m in MLPS}
    aw_d = {k: nc.dram_tensor(f"aw_{k}", [P, D], F32, kind="ExternalInput") for k in AW_KEYS}
    b_d = {m: nc.dram_tensor(f"b_{m}", [D], F32, kind="ExternalInput") for m in MLPS}
    g_d = {m: nc.dram_tensor(f"g_{m}", [D], F32, kind="ExternalInput") for m in MLPS}
    bt_d = {m: nc.dram_tensor(f"bt_{m}", [D], F32, kind="ExternalInput") for m in MLPS}
    outs = {
        name: nc.dram_tensor(name, [T, D], F32, kind="ExternalOutput")
        for name in ["o_common", "o_synergy", "o_gspec", "o_pspec"]
    }

    with TileContext(nc) as tc:
        with (
            tc.tile_pool(name="consts", bufs=1) as consts,
            tc.tile_pool(name="xnat", bufs=4) as xnat,
            tc.tile_pool(name="work", bufs=14) as work,
            tc.tile_pool(name="spool", bufs=14) as spool,
            tc.tile_pool(name="tpsum", bufs=2, space="PSUM") as tpsum,
            tc.tile_pool(name="hpsum", bufs=6, space="PSUM") as hpsum,
        ):
            ident = consts.tile([P, P], F32)
            make_identity(nc, ident)
            eps_t = consts.tile([P, 1], F32)
            nc.vector.memset(eps_t[:], LN_EPS)
            abt = {}
            for k in AW_KEYS:
                abt[k] = consts.tile([P, 1], F32, tag=f"ab_{k}", name=f"ab_{k}")
                nc.vector.memset(abt[k][:], ab[k])

            wt_t = {}
            for m in MLPS:
                wt_t[m] = consts.tile([P, NK, D], F32, tag=f"wt_{m}", name=f"wt_{m}")
                nc.sync.dma_start(
                    out=wt_t[m][:],
                    in_=wt_d[m][:].rearrange("(kb p) j -> p kb j", p=P),
                )
            awbc = {}
            for k in AW_KEYS:
                awbc[k] = consts.tile([P, D], F32, tag=f"aw_{k}", name=f"aw_{k}")
                nc.sync.dma_start(out=awbc[k][:], in_=aw_d[k][:])

            ones_t = consts.tile([1, P], F32, tag="ones")
            nc.vector.memset(ones_t[:], 1.0)
            b_t, gbc, btbc = {}, {}, {}
            for m in MLPS:
                b_t[m] = consts.tile([1, D], F32, tag=f"b_{m}", name=f"b_{m}")
                nc.sync.dma_start(out=b_t[m][:], in_=b_d[m][:].rearrange("d -> 1 d"))
                gbc[m] = consts.tile([P, D], F32, tag=f"g_{m}", name=f"g_{m}")
                nc.gpsimd.dma_start(out=gbc[m][:], in_=_bcast_rows(g_d[m][:], P))
                btbc[m] = consts.tile([P, D], F32, tag=f"bt_{m}", name=f"bt_{m}")
                nc.gpsimd.dma_start(out=btbc[m][:], in_=_bcast_rows(bt_d[m][:], P))

            xt = {}
            for inp in ("g", "p"):
                xt[inp] = consts.tile([P, NK, T], F32, tag=f"xt_{inp}", name=f"xt_{inp}")
                for nb in range(NT):
                    xn = xnat.tile([P, D], F32, tag="xn")
                    nc.sync.dma_start(out=xn[:], in_=xin[inp][nb * P:(nb + 1) * P, :])
                    for kb in range(NK):
                        tp = tpsum.tile([P, P], F32, tag="tp")
                        nc.tensor.transpose(tp[:], xn[:, kb * P:(kb + 1) * P], ident[:])
                        nc.vector.tensor_copy(out=xt[inp][:, kb, nb * P:(nb + 1) * P], in_=tp[:])

            hp = {}
            for inp in ("g", "p"):
                for kb in range(NK):
                    for nb in range(NT):
                        tok = slice(nb * P, (nb + 1) * P)
                        for m in MLP_BY_INP[inp]:
                            if kb == 0 and nb == 0:
                                hp[m] = hpsum.tile([P, NT, D], F32, tag="hp", name=f"hp_{m}")
                            nc.tensor.matmul(
                                hp[m][:, nb, :],
                                lhsT=xt[inp][:, kb, tok],
                                rhs=wt_t[m][:, kb, :],
                                start=(kb == 0 and nb == 0),
                                stop=False,
                            )
                for nb in range(NT):
                    for m in MLP_BY_INP[inp]:
                        nc.tensor.matmul(
                            hp[m][:, nb, :],
                            lhsT=ones_t[0:1, :],
                            rhs=b_t[m][0:1, :],
                            start=False,
                            stop=(nb == NT - 1),
                        )

            TILES = [(m, nb) for nb in range(NT) for m in LN_ORDER]
            mva = spool.tile([P, 12, 2], F32, tag="mva")
            for i, (m, nb) in enumerate(TILES):
                stats = spool.tile([P, 6], F32, tag="stats")
                nc.vector.bn_stats(stats[:], hp[m][:, nb, :])
                nc.vector.bn_aggr(mva[:, i, :], stats[:])
            stdall = spool.tile([P, 12], F32, tag="stdall")
            nc.scalar.activation(stdall[:], mva[:, :, 1], AF.Sqrt, bias=eps_t[:])
            rstdall = spool.tile([P, 12], F32, tag="rstdall")
            nc.vector.reciprocal(rstdall[:], stdall[:])
            nmrall = spool.tile([P, 12], F32, tag="nmrall")
            nc.vector.tensor_mul(nmrall[:], mva[:, :, 0], rstdall[:])
            nc.vector.tensor_scalar(nmrall[:], nmrall[:], scalar1=-1.0, scalar2=None, op0=ALU.mult)

            aligns = {}
            for i, (m, nb) in enumerate(TILES):
                is_align = m not in ("gs", "ps")
                otag = "align" if is_align else "spec"
                ot = work.tile([P, D], F32, tag=otag, name=f"ot_{m}_{nb}")
                nc.scalar.activation(ot[:], hp[m][:, nb, :], AF.Identity,
                                     bias=nmrall[:, i:i + 1], scale=rstdall[:, i:i + 1])
                nc.vector.tensor_mul(ot[:], ot[:], gbc[m][:])
                nc.vector.tensor_add(ot[:], ot[:], btbc[m][:])
                nc.vector.tensor_scalar_max(ot[:], ot[:], 0.0)
                tok = slice(nb * P, (nb + 1) * P)
                if m == "gs":
                    nc.sync.dma_start(out=outs["o_gspec"][tok, :], in_=ot[:])
                elif m == "ps":
                    nc.sync.dma_start(out=outs["o_pspec"][tok, :], in_=ot[:])
                else:
                    aligns[(m, nb)] = ot

            dots = {}
            for nb in range(NT):
                for pr in ("c", "s"):
                    gal = aligns[(pr + "_g", nb)]
                    pal = aligns[(pr + "_p", nb)]
                    sc1 = work.tile([P, D], F32, tag="ttscratch")
                    dp = spool.tile([P, 1], F32, tag="dp", name=f"dp_{pr}_{nb}")
                    nc.vector.tensor_mul(sc1[:], pal[:], awbc[pr + "_g"][:])
                    nc.vector.tensor_reduce(dp[:], sc1[:], axis=mybir.AxisListType.X, op=ALU.add)
                    sc2 = work.tile([P, D], F32, tag="ttscratch")
                    dg = spool.tile([P, 1], F32, tag="dg", name=f"dg_{pr}_{nb}")
                    nc.vector.tensor_mul(sc2[:], gal[:], awbc[pr + "_p"][:])
                    nc.vector.tensor_reduce(dg[:], sc2[:], axis=mybir.AxisListType.X, op=ALU.add)
                    dots[(pr, nb)] = (dp, dg)
            for nb in range(NT):
                for pr, oname in (("c", "o_common"), ("s", "o_synergy")):
                    gal = aligns[(pr + "_g", nb)]
                    pal = aligns[(pr + "_p", nb)]
                    dp, dg = dots[(pr, nb)]
                    gat = work.tile([P, D], F32, tag="att")
                    nc.scalar.activation(gat[:], gal[:], AF.Sigmoid, bias=abt[pr + "_g"][:], scale=dp[:])
                    pat = work.tile([P, D], F32, tag="att")
                    nc.scalar.activation(pat[:], pal[:], AF.Sigmoid, bias=abt[pr + "_p"][:], scale=dg[:])
                    t1 = work.tile([P, D], F32, tag="t1")
                    nc.gpsimd.tensor_mul(t1[:], pal[:], pat[:])
                    t2 = work.tile([P, D], F32, tag="t2")
                    nc.gpsimd.tensor_mul(t2[:], gal[:], gat[:])
                    ot2 = work.tile([P, D], F32, tag="iout")
                    nc.vector.tensor_add(ot2[:], t1[:], t2[:])
                    tok = slice(nb * P, (nb + 1) * P)
                    nc.sync.dma_start(out=outs[oname][tok, :], in_=ot2[:])

    nc.compile()
    return nc


_CACHE: dict = {}


def _prepare(inputs: dict):
    """Returns (nc, in_maps) for the given full inputs."""
    inp = {k: np.asarray(v) for k, v in inputs.items()}
    gfeat = np.ascontiguousarray(inp["gfeat"], dtype=np.float32)
    pfeat = np.ascontiguousarray(inp["pfeat"], dtype=np.float32)

    affine_identity = all(
        (inp[m + "_b"] == 0).all()
        and (inp[m + "_g"] == 1).all()
        and (inp[m + "_beta"] == 0).all()
        for m in MLPS
    )

    if affine_identity:
        # slot order [c_p, s_p, c_g, s_g]: biases (c_apb, s_apb, c_agb, s_agb)
        ab4 = (float(inp["c_apb"]), float(inp["s_apb"]),
               float(inp["c_agb"]), float(inp["s_agb"]))
        key = ("fast", ab4)
        if key not in _CACHE:
            _CACHE[key] = _build_fast(ab4)
        nc = _CACHE[key]

        xsh = {"g": gfeat.reshape(B * L, D), "p": pfeat.reshape(B * L, D)}
        base = {}
        for s in ("g", "p"):
            # merged, centered rhs: concat over SLAB_ORDER of (W.T - colmean)
            cols = []
            for m in SLAB_ORDER[s]:
                wt = inp[f"{m}_W"].astype(np.float32).T          # [k, j]
                wc = wt - wt.mean(axis=1, keepdims=True)
                cols.append(wc)
            rhs = np.concatenate(cols, axis=1)                    # [256, 768]
            rhs = rhs.reshape(NK, P, W3).transpose(1, 0, 2)       # [P, NK, 768]
            base[f"rhs_{s}"] = np.ascontiguousarray(
                rhs.reshape(P, NK * W3).astype(ml_dtypes.bfloat16))
        aw = np.stack([
            inp["c_agw"], inp["s_agw"], inp["c_apw"], inp["s_apw"]
        ]).astype(np.float32)                                     # [4, D]
        aw4 = np.broadcast_to(aw.reshape(1, 4 * D), (P, 4 * D))
        base["aw4"] = np.ascontiguousarray(aw4.astype(ml_dtypes.bfloat16))

        in_maps = []
        for c in range(N_CORES):
            m = dict(base)
            for s in ("g", "p"):
                xc = xsh[s][c * T:(c + 1) * T, :]                 # [T, D]
                xt = xc.reshape(T, NK, P).transpose(2, 1, 0)      # [P, NK, T]
                m[f"xt_{s}"] = np.ascontiguousarray(
                    xt.reshape(P, NK * T).astype(ml_dtypes.bfloat16))
            in_maps.append(m)
        return nc, in_maps

    # ---- generic fallback ----
    ab = {
        "c_g": float(inp["c_agb"]),
        "c_p": float(inp["c_apb"]),
        "s_g": float(inp["s_agb"]),
        "s_p": float(inp["s_apb"]),
    }
    key = ("generic", tuple(sorted(ab.items())))
    if key not in _CACHE:
        _CACHE[key] = _build_generic(ab)
    nc = _CACHE[key]

    base = {
        f"wt_{m}": np.ascontiguousarray(inp[f"{m}_W"].T, dtype=np.float32)
        for m in MLPS
    }
    base["aw_c_g"] = np.ascontiguousarray(np.broadcast_to(inp["c_agw"].astype(np.float32), (P, D)))
    base["aw_c_p"] = np.ascontiguousarray(np.broadcast_to(inp["c_apw"].astype(np.float32), (P, D)))
    base["aw_s_g"] = np.ascontiguousarray(np.broadcast_to(inp["s_agw"].astype(np.float32), (P, D)))
    base["aw_s_p"] = np.ascontiguousarray(np.broadcast_to(inp["s_apw"].astype(np.float32), (P, D)))
    for m in MLPS:
        base[f"b_{m}"] = np.ascontiguousarray(inp[f"{m}_b"], dtype=np.float32)
        base[f"g_{m}"] = np.ascontiguousarray(inp[f"{m}_g"], dtype=np.float32)
        base[f"bt_{m}"] = np.ascontiguousarray(inp[f"{m}_beta"], dtype=np.float32)

    gsh = gfeat.reshape(N_CORES, T, D)
    psh = pfeat.reshape(N_CORES, T, D)
    in_maps = [dict(base, xg=gsh[c], xp=psh[c]) for c in range(N_CORES)]
    return nc, in_maps


def kernel(**inputs) -> tuple:
    nc, in_maps = _prepare(inputs)
    res = run_bass_kernel_spmd(nc, in_maps, list(range(N_CORES)))

    def gather(name):
        return np.concatenate(
            [res.results[c][name].reshape(BS, L, D) for c in range(N_CORES)], axis=0
        )

    return (gather("o_common"), gather("o_synergy"), gather("o_gspec"), gather("o_pspec"))
